# revision 1
# baseline (speedup 1.0000x reference)
"""Trainium2 Bass/Tile kernel for nn_CrossAttention_54434415509663.

Sharding: 8 cores = 2 batches x 4 row-shards. Core c handles batch c//4,
row-quarter c%4, with halo recompute for conv receptive fields. BatchNorm
batch statistics via AllGather of per-channel (sum, sumsq) + local sum
(cheaper than AllReduce for tiny payloads); windowed cross-attention k/v
via per-batch AllGather (replica groups of 4).

v2: conv path in bf16 (inputs, weights, activations, residual, output)
to halve DMA/SBUF; attention in float32r. The x / context reduce stacks
are software-pipelined one stage apart so each BN stats collective hides
behind the other tensor's conv. Residual rows prefetch during tconv3;
final residual add runs on the idle Pool engine.

SBUF/PSUM are allocated with a fixed slot plan: the Tile allocator is
static per (pool, tag), so sequential-lifetime buffers share tags.

Self-contained: accepts FULL inputs, returns FULL output.
"""
import sys
from contextlib import ExitStack

import numpy as np
import ml_dtypes

NPBF = ml_dtypes.bfloat16

sys.path.insert(0, "/opt/trn_rl_repo")

import concourse.bass as bass  # noqa: E402
import concourse.bacc as bacc  # noqa: E402
import concourse.tile as tile  # noqa: E402
from concourse import mybir  # noqa: E402
from concourse.bass_utils import run_bass_kernel_spmd  # noqa: E402

P = 128
C = 256
KT = C // P          # channel tiles (contraction)
MT = C // P          # channel tiles (output)
HEADS = 8
HD = C // HEADS
WS = 16
EPS = 1e-5
NCORES = 8
NSH = 4              # row shards per batch
F32 = mybir.dt.float32
F32R = mybir.dt.float32r
BF16 = mybir.dt.bfloat16
AF = mybir.ActivationFunctionType


def _cdiv(a, b):
    return -(-a // b)


def tconv_taps(pa):
    """(ky, dy) pairs for output abs parity pa (jax lhs_dilated conv k4 s2 p2:
    out[y] += in[(y+ky-2)/2] * w[ky])."""
    return ((0, -1), (2, 0)) if pa == 0 else ((1, 0), (3, 1))


class Geo:
    """Row extents per stage. abs range for shard s = [al*s+lo, al*s+hi] incl."""

    def __init__(self, H, W):
        assert H % 32 == 0 and W % 32 == 0
        self.H, self.W = H, W
        self.W1, self.W2, self.Wr = W // 2, W // 4, W // 8
        self.H1, self.H2, self.Hr = H // 2, H // 4, H // 8
        self.N = self.Hr * self.Wr          # tokens per batch
        u3 = (H // 4, 0, H // 4 - 1)
        u2 = self._tin(u3)
        u1 = self._tin(u2)
        bott = self._tin(u1)
        r3 = bott
        r2 = self._cin(r3)
        r1 = self._cin(r2)
        r0 = self._cin(r1)
        self.u3, self.u2, self.u1 = u3, u2, u1
        self.bott, self.r3, self.r2, self.r1, self.r0 = bott, r3, r2, r1, r0
        self.n = {k: v[2] - v[1] + 1
                  for k, v in dict(u3=u3, u2=u2, u1=u1, bott=bott,
                                   r2=r2, r1=r1, r0=r0).items()}
        self.n['r3'] = self.n['bott']
        self.E = self.n['bott'] * self.Wr    # ext tokens per core
        self.T = self.N // NSH               # owned tokens per core
        self.KJT = min(128, self.N)
        assert self.N % self.KJT == 0
        self.NT = self.N // self.KJT
        assert self.E <= 512

    @staticmethod
    def _tin(rng):
        al, lo, hi = rng
        return al // 2, -(-(lo - 2) // 2), (hi + 1) // 2

    @staticmethod
    def _cin(rng):
        al, lo, hi = rng
        return al * 2, 2 * lo - 1, 2 * hi + 1

    def owned_local(self, st):
        return -getattr(self, st)[1]

    def bands(self, st, Hst):
        al, lo, hi = getattr(self, st)
        return -lo, al * (NSH - 1) + hi - (Hst - 1)

    def tconv_ybase(self, out_st, in_st):
        """per local row class r: (pa, ybase) with in_local = t + ybase + dy;
        asserts shard-independence."""
        al, lo, hi = getattr(self, out_st)
        ial, ilo, ihi = getattr(self, in_st)
        res = []
        for r in range(2):
            pa = (lo + r) & 1
            vals = set()
            for s in range(NSH):
                y0 = al * s + lo + r
                m = (y0 - pa) // 2
                vals.add(m - (ial * s + ilo))
            assert len(vals) == 1, (out_st, r, vals)
            res.append((pa, vals.pop()))
        return res


def query_kv_ranges(Hr, Wr):
    N = Hr * Wr
    rng = [None] * N
    for hi_ in range(_cdiv(Hr, WS)):
        for wi in range(_cdiv(Wr, WS)):
            s = hi_ * WS * Wr + wi * WS
            e = min(min(hi_ * WS + WS, Hr) * Wr + min(wi * WS + WS, Wr), N)
            for t in range(s, e):
                rng[t] = (s, e)
    return rng


# out-of-image band widths (top, bot) per masked stage; H-independent
BANDS = {'r1': (7, 4), 'r2': (3, 2), 'bott': (1, 1), 'u1': (1, 1),
         'u2': (1, 1)}


def _aux_cols():
    cols = {}
    i = 0
    for name in ('red_g', 'red_beta', 'up_g', 'up_beta'):
        for st in range(3):
            for kt in range(KT):
                cols[(name, st, kt)] = i
                i += 1
    for name in ('bq', 'bv', 'bp'):
        for mt in range(MT):
            cols[(name, mt)] = i
            i += 1
    for name, (tb, bb) in BANDS.items():
        for r in range(tb):
            cols[(name, 't', r)] = i
            i += 1
        for r in range(bb):
            cols[(name, 'b', r)] = i
            i += 1
    cols['ones'] = i
    return cols, i + 1


AUX, NAUX = _aux_cols()


# ---------------------------------------------------------------- builder
def build(H, W, dbg=False):
    g = Geo(H, W)
    nc = bacc.Bacc("TRN2", target_bir_lowering=False, debug=False,
                   num_devices=NCORES)

    xin = nc.dram_tensor("xin", [KT, P, g.n['r0'], W + 2], BF16,
                         kind="ExternalInput").ap()
    cin = nc.dram_tensor("cin", [KT, P, g.n['r0'], W + 2], BF16,
                         kind="ExternalInput").ap()
    wred_d = nc.dram_tensor("wred", [3, 9 * KT * MT, P, P], BF16,
                            kind="ExternalInput").ap()
    wup_d = nc.dram_tensor("wup", [3, 4, 4 * KT * MT, P, P], BF16,
                           kind="ExternalInput").ap()
    wq_d = nc.dram_tensor("wq_t", [KT * MT, P, P], F32,
                          kind="ExternalInput").ap()
    wk_d = nc.dram_tensor("wk_t", [KT * MT, P, P], F32,
                          kind="ExternalInput").ap()
    wv_d = nc.dram_tensor("wv_t", [KT, P, C], F32, kind="ExternalInput").ap()
    wp_d = nc.dram_tensor("wp_t", [KT * MT, P, P], F32,
                          kind="ExternalInput").ap()
    aux_d = nc.dram_tensor("aux", [P, NAUX], F32, kind="ExternalInput").ap()
    ones_row_d = nc.dram_tensor("ones_row", [1, P], F32,
                                kind="ExternalInput").ap()
    onesc_d = nc.dram_tensor("onesc", [P, 1], F32, kind="ExternalInput").ap()
    zpad_d = nc.dram_tensor("zpad", [P, P], BF16, kind="ExternalInput").ap()
    kmask_d = nc.dram_tensor("kmask", [g.NT, g.KJT, g.E], BF16,
                             kind="ExternalInput").ap()
    out_d = nc.dram_tensor("out", [KT, P, g.n['u3'], W], BF16,
                           kind="ExternalOutput").ap()
    def tap(name, tiles):
        if not dbg:
            return
        sh = list(tiles[0].shape)
        d = nc.dram_tensor("dbg_" + name, [len(tiles)] + sh,
                           tiles[0].dtype, kind="ExternalOutput").ap()
        for i, t in enumerate(tiles):
            nc.sync.dma_start(out=d[i], in_=t[:])

    att_scale = float(HD) ** -0.5
    n1, n2, n3, nb = g.n['r1'], g.n['r2'], g.n['r3'], g.n['bott']
    nu1, nu2, nu3 = g.n['u1'], g.n['u2'], g.n['u3']
    nu3h = nu3 // 2

    with tile.TileContext(nc) as tc, ExitStack() as ctx:
        mega = ctx.enter_context(tc.tile_pool(name="mega", bufs=1))
        work = ctx.enter_context(tc.tile_pool(name="work", bufs=2))
        statp = ctx.enter_context(tc.tile_pool(name="statp", bufs=2))
        stat1 = ctx.enter_context(tc.tile_pool(name="stat1", bufs=1))
        psp = ctx.enter_context(tc.tile_pool(name="psp", bufs=1,
                                             space="PSUM"))
        dram = ctx.enter_context(tc.tile_pool(name="dram", bufs=2,
                                              space="DRAM"))

        # ---- constants
        aux = mega.tile([P, NAUX], F32, tag="aux")
        nc.sync.dma_start(out=aux[:], in_=aux_d)
        ones_row = mega.tile([1, P], F32, tag="ones_row")
        nc.sync.dma_start(out=ones_row[:].bitcast(F32R),
                          in_=ones_row_d.bitcast(F32R))
        eps_t = mega.tile([P, 1], F32, tag="eps")
        nc.vector.memset(eps_t[:], EPS)
        ones_col = mega.tile([P, 1], F32, tag="onesc")
        nc.sync.dma_start(out=ones_col[:].bitcast(F32R),
                          in_=onesc_d.bitcast(F32R))
        ones_col = ones_col[:]
        ones_col_b = mega.tile([P, 1], BF16, tag="onescb")
        nc.scalar.copy(out=ones_col_b[:], in_=ones_col)
        kmask = mega.tile([g.KJT, g.NT, g.E], BF16, tag="kmask")
        nc.sync.dma_start(out=kmask[:],
                          in_=kmask_d.rearrange("t p e -> p t e"))

        def acol(*key):
            i = AUX[key if len(key) > 1 else key[0]]
            return aux[:, i:i + 1]

        _mm_ctr = [0]

        def mm_tag():
            _mm_ctr[0] += 1
            return f"mm{_mm_ctr[0] % 4}"

        def mm_group(ps_ap, pairs):
            n = len(pairs)
            for i, (l, r) in enumerate(pairs):
                nc.tensor.matmul(ps_ap, l.bitcast(F32R), r.bitcast(F32R),
                                 start=(i == 0), stop=(i == n - 1))

        # ---------------------------------------------------- BN helpers
        def bn_stats_pack(own_aps, kt, n_loc, pack, tg=""):
            """own_aps: list of [P, rows, cols] APs; (sum, sumsq) -> pack.

            bn_stats: each middle-dim row of a [P, r, c] input is one group
            (r*c <= 512 per call); bn_aggr combines all row-groups."""
            G = sum(ap.shape[1] for ap in own_aps)
            st = statp.tile([P, G, 6], F32, tag="bnst" + tg[-1])
            o = 0
            for ap in own_aps:
                for r in range(ap.shape[1]):
                    nc.vector.bn_stats(out=st[:, o, :], in_=ap[:, r, :])
                    o += 1
            mv = statp.tile([P, 2], F32, tag="bnmv")
            nc.vector.bn_aggr(out=mv[:], in_=st[:])
            t1 = work.tile([P, 1], F32, tag="bn_t1")
            nc.vector.tensor_mul(t1[:], mv[:, 0:1], mv[:, 0:1])
            nc.vector.tensor_add(t1[:], t1[:], mv[:, 1:2])
            nc.scalar.mul(out=pack[:, kt:kt + 1], in_=mv[:, 0:1],
                          mul=float(n_loc))
            nc.scalar.mul(out=pack[:, KT + kt:KT + kt + 1], in_=t1[:],
                          mul=float(n_loc))

        def stats_send(pack, tg):
            """AllGather the [P, 2KT] (sum, sumsq) pack across all cores."""
            bi = dram.tile([P, 2 * KT], F32, tag="ag_i" + tg)
            bo = dram.tile([NCORES * P, 2 * KT], F32, tag="ag_o" + tg)
            nc.sync.dma_start(out=bi[:], in_=pack[:])
            nc.gpsimd.collective_compute(
                "AllGather", mybir.AluOpType.bypass,
                replica_groups=[list(range(NCORES))],
                ins=[bi.opt()], outs=[bo.opt()])
            return bo

        def stats_recv(bo, n_glob, gkey, stg):
            sb = statp.tile([P, 2 * KT, NCORES], F32, tag="ar_sb")
            nc.sync.dma_start(
                out=sb[:], in_=bo.rearrange("(gr p) c -> p c gr", p=P))
            s8 = statp.tile([P, 2 * KT], F32, tag="ar_s8")
            nc.vector.tensor_reduce(out=s8[:], in_=sb[:],
                                    axis=mybir.AxisListType.X,
                                    op=mybir.AluOpType.add)
            m = statp.tile([P, KT], F32, tag="bn_m")
            v = statp.tile([P, KT], F32, tag="bn_v")
            nc.scalar.mul(out=m[:], in_=s8[:, 0:KT], mul=1.0 / n_glob)
            nc.scalar.mul(out=v[:], in_=s8[:, KT:2 * KT], mul=1.0 / n_glob)
            msq = work.tile([P, KT], F32, tag="bn_msq")
            nc.vector.tensor_mul(msq[:], m[:], m[:])
            nc.vector.tensor_sub(v[:], v[:], msq[:])
            nc.scalar.activation(out=v[:], in_=v[:], func=AF.Sqrt,
                                 bias=eps_t[:])
            nc.vector.reciprocal(v[:], v[:])      # rstd
            a = statp.tile([P, KT], F32, tag="bn_a")
            b = statp.tile([P, KT], F32, tag="bn_b")
            gg = work.tile([P, KT], F32, tag="bn_gg")
            for kt in range(KT):
                nc.vector.tensor_copy(gg[:, kt:kt + 1], acol(gkey, stg, kt))
            nc.vector.tensor_mul(a[:], v[:], gg[:])
            nc.vector.tensor_mul(m[:], m[:], a[:])
            bkey = gkey.replace('_g', '_beta')
            for kt in range(KT):
                nc.vector.tensor_copy(gg[:, kt:kt + 1], acol(bkey, stg, kt))
            nc.vector.tensor_sub(b[:], gg[:], m[:])
            return a, b

        def bn_apply(tiles_per_kt, interior, a, b):
            for kt in range(KT):
                for ap in interior(kt):
                    nc.scalar.activation(out=ap, in_=ap,
                                         func=AF.Relu,
                                         scale=a[:, kt:kt + 1],
                                         bias=b[:, kt:kt + 1])

        def edge_mask(tiles, mname, bands):
            tb, bb = bands
            assert (tb, bb) == BANDS[mname]
            for t in tiles:
                nrows = t.shape[1]
                for r in range(tb):
                    nc.vector.tensor_scalar_mul(
                        out=t[:, r:r + 1, :],
                        in0=t[:, r:r + 1, :],
                        scalar1=acol(mname, 't', r))
                for r in range(bb):
                    rr = nrows - bb + r
                    nc.vector.tensor_scalar_mul(
                        out=t[:, rr:rr + 1, :],
                        in0=t[:, rr:rr + 1, :],
                        scalar1=acol(mname, 'b', r))

        def alloc_padded(tags, n_rows, wcols, pad, dt=BF16):
            tiles = []
            for kt in range(KT):
                t = mega.tile([P, n_rows, wcols + 2 * pad], dt, tag=tags[kt])
                if pad:
                    zsrc = zpad_d[:, 0:n_rows].rearrange(
                        "p (n o) -> p n o", o=1)
                    nc.sync.dma_start(out=t[:, :, 0:pad], in_=zsrc)
                    nc.sync.dma_start(out=t[:, :, wcols + pad:], in_=zsrc)
                tiles.append(t)
            return tiles

        # ---------------------------------------------------- conv stages
        def copy_act(out, in_):
            nc.scalar.copy(out=out, in_=in_)

        def copy_dve(out, in_):
            nc.vector.tensor_copy(out=out, in_=in_)

        def mm_group_bf(ps_ap, pairs):
            n = len(pairs)
            for i, (l, r) in enumerate(pairs):
                nc.tensor.matmul(ps_ap, l, r,
                                 start=(i == 0), stop=(i == n - 1))

        def conv_from_dram(src_d, wt, n_out, Wo, out_tiles):
            rpb = max(1, 512 // Wo)
            y0 = 0
            blki = 0
            while y0 < n_out:
                nr = min(rpb, n_out - y0)
                blk = []
                for kt in range(KT):
                    t = mega.tile([P, 2 * rpb + 1, W + 2], BF16,
                                  tag=f"T{2 * kt + blki % 2}")
                    nc.sync.dma_start(
                        out=t[:, 0:2 * nr + 1, :],
                        in_=src_d[kt, :, 2 * y0:2 * y0 + 2 * nr + 1, :])
                    blk.append(t)
                for mt in range(MT):
                    ps = psp.tile([P, rpb, Wo], F32, tag=mm_tag())
                    pairs = []
                    for kt in range(KT):
                        for ky in range(3):
                            for kx in range(3):
                                idx = ((ky * 3 + kx) * KT + kt) * MT + mt
                                rhs = blk[kt][:, ky:ky + 2 * nr - 1:2,
                                              kx:kx + 2 * Wo - 1:2]
                                pairs.append((wt[:, idx, :], rhs))
                    mm_group_bf(ps[:, 0:nr, :], pairs)
                    ce = copy_act if mt == 0 else copy_dve
                    ce(out=out_tiles[mt][:, y0:y0 + nr, 1:Wo + 1],
                       in_=ps[:, 0:nr, :])
                y0 += nr
                blki += 1

        def conv_resident(src, wt, n_out, Wo, out_tiles, out_pad):
            rpb = max(1, 512 // Wo)
            y0 = 0
            while y0 < n_out:
                nr = min(rpb, n_out - y0)
                for mt in range(MT):
                    ps = psp.tile([P, rpb, Wo], F32, tag=mm_tag())
                    pairs = []
                    for kt in range(KT):
                        for ky in range(3):
                            for kx in range(3):
                                idx = ((ky * 3 + kx) * KT + kt) * MT + mt
                                rhs = src[kt][:, 2 * y0 + ky:
                                              2 * y0 + ky + 2 * nr - 1:2,
                                              kx:kx + 2 * Wo - 1:2]
                                pairs.append((wt[:, idx, :], rhs))
                    mm_group_bf(ps[:, 0:nr, :], pairs)
                    if out_pad:
                        dst = out_tiles[mt][:, y0:y0 + nr, 1:Wo + 1]
                    else:
                        dst = out_tiles[mt][:, y0:y0 + nr, :]
                    ce = copy_act if mt == 0 else copy_dve
                    ce(out=dst, in_=ps[:, 0:nr, :])
                y0 += nr

        # ---------------------------------------------------- reduce stacks
        def load_wconv(st):
            wt = mega.tile([P, 9 * KT * MT, P], BF16, tag=f"wA{st % 2}")
            nc.sync.dma_start(out=wt[:],
                              in_=wred_d[st].rearrange("t p q -> p t q"))
            return wt

        # x / c stacks staggered one stage apart: each stats AllGather
        # overlaps the other tensor's conv of the same stage.
        RED = ((n1, g.W1, g.H1, 'r1', 1), (n2, g.W2, g.H2, 'r2', 1),
               (n3, g.Wr, g.Hr, 'r3', 0))

        def red_stage(src, wt, stage, tags, from_dram, defer_send=False):
            n_out, Wo, Hst, stname, pad = RED[stage]
            tiles = alloc_padded(tags, n_out, Wo, pad)
            if from_dram:
                conv_from_dram(src, wt, n_out, Wo, tiles)
            else:
                conv_resident(src, wt, n_out, Wo, tiles, pad == 1)
            pk = statp.tile([P, 2 * KT], F32, tag="pack" + tags[0])
            ol = g.owned_local(stname)
            for kt in range(KT):
                ap = tiles[kt][:, ol:ol + Hst // NSH, pad:Wo + pad]
                bn_stats_pack([ap], kt, (Hst // NSH) * Wo, pk, tg=tags[0])
            if defer_send:
                return tiles, pk
            bo = stats_send(pk, tags[0])
            return tiles, bo

        def red_fin(tiles, bo, stage):
            n_out, Wo, Hst, stname, pad = RED[stage]
            a, b = stats_recv(bo, 2 * Hst * Wo, 'red_g', stage)
            bn_apply(tiles, lambda kt: [tiles[kt][:, :, pad:Wo + pad]], a, b)
            if stage < 2:
                edge_mask(tiles, stname, g.bands(stname, Hst))

        w0 = load_wconv(0)
        b1c, agc1 = red_stage(cin, w0, 0, ("bigC0", "bigC1"), True)
        b1x, agx1 = red_stage(xin, w0, 0, ("bigX0", "bigX1"), True)
        w1 = load_wconv(1)
        red_fin(b1c, agc1, 0)
        b2c, agc2 = red_stage(b1c, w1, 1, ("midC0", "midC1"), False)
        red_fin(b1x, agx1, 0)
        b2x, agx2 = red_stage(b1x, w1, 1, ("midX0", "midX1"), False)
        w2 = load_wconv(2)
        red_fin(b2c, agc2, 1)
        c3, agc3 = red_stage(b2c, w2, 2, ("sm2", "sm3"), False)
        red_fin(b2x, agx2, 1)
        x3, x3pk = red_stage(b2x, w2, 2, ("sm0", "sm1"), False,
                             defer_send=True)

        # ---------------------------------------------------- attention
        def ln_norm(src_flat, ztags, bf=False):
            E = src_flat[0].shape[-1]
            oc = ones_col_b[:] if bf else ones_col.bitcast(F32R)
            sums = psp.tile([1, E], F32, tag="att0")
            for kt in range(KT):
                src = src_flat[kt] if bf else src_flat[kt].bitcast(F32R)
                nc.tensor.matmul(sums[:], oc, src,
                                 start=(kt == 0), stop=(kt == KT - 1))
            sqs = psp.tile([1, E], F32, tag="att1")
            for kt in range(KT):
                scr = work.tile([P, E], F32, tag="ln_scr")
                nc.scalar.square(out=scr[:].bitcast(F32R), in_=src_flat[kt])
                nc.tensor.matmul(sqs[:], ones_col.bitcast(F32R),
                                 scr[:].bitcast(F32R),
                                 start=(kt == 0), stop=(kt == KT - 1))
            m = stat1.tile([1, E], F32, tag="ln_m")
            v = stat1.tile([1, E], F32, tag="ln_v")
            nc.scalar.mul(out=m[:].bitcast(F32R), in_=sums[:], mul=1.0 / C)
            nc.scalar.mul(out=v[:].bitcast(F32R), in_=sqs[:],
                          mul=1.0 / C)
            t = stat1.tile([1, E], F32, tag="ln_t")
            nc.vector.tensor_mul(t[:], m[:], m[:])
            nc.vector.tensor_sub(v[:].bitcast(F32R), v[:], t[:])
            nc.scalar.activation(out=v[:].bitcast(F32R), in_=v[:],
                                 func=AF.Sqrt, bias=eps_t[0:1, :])
            with nc.allow_low_precision(reason="f32r feed to PE broadcast"):
                nc.vector.reciprocal(v[:].bitcast(F32R), v[:])
            mb = psp.tile([P, E], F32, tag="attS")
            rb = psp.tile([P, E], F32, tag="attX")
            nc.tensor.matmul(mb[:], ones_row[:].bitcast(F32R),
                             m[:].bitcast(F32R), start=True, stop=True)
            nc.tensor.matmul(rb[:], ones_row[:].bitcast(F32R),
                             v[:].bitcast(F32R), start=True, stop=True)
            zs = []
            for kt in range(KT):
                z = mega.tile([P, E], F32, tag=ztags[kt])
                if bf:
                    nc.scalar.copy(out=z[:].bitcast(F32R),
                                   in_=src_flat[kt])
                    nc.vector.tensor_sub(z[:].bitcast(F32R), z[:], mb[:])
                else:
                    nc.vector.tensor_sub(z[:].bitcast(F32R),
                                         src_flat[kt], mb[:])
                nc.vector.tensor_mul(z[:].bitcast(F32R), z[:], rb[:])
                zs.append(z)
            return zs

        wq_t = mega.tile([P, KT * MT, P], F32, tag="wa1")
        wk_t = mega.tile([P, KT * MT, P], F32, tag="wk")
        wv_t = mega.tile([P, KT, C], F32, tag="wv")
        nc.sync.dma_start(out=wq_t[:].bitcast(F32R),
                          in_=wq_d.rearrange("t p q -> p t q").bitcast(F32R))
        nc.sync.dma_start(out=wk_t[:].bitcast(F32R),
                          in_=wk_d.rearrange("t p q -> p t q").bitcast(F32R))
        nc.sync.dma_start(out=wv_t[:].bitcast(F32R),
                          in_=wv_d.rearrange("t p q -> p t q").bitcast(F32R))

        red_fin(c3, agc3, 2)
        c3f = [t[:].rearrange("p a b -> p (a b)") for t in c3]
        zc = ln_norm(c3f, ("z2", "z3"), bf=True)
        tap("zc", zc)

        # local k (ch-layout) / v (tok-layout) on owned tokens (bf16), then
        # AllGather; the gather overlaps the x-side LN + q projection.
        T = g.T
        kv_b = dram.tile([C + T, C], BF16, tag="kv_in")
        tok0 = g.Wr
        for mt in range(MT):
            ps = psp.tile([P, T], F32, tag=mm_tag())
            mm_group(ps[:], [(wk_t[:, kt * MT + mt, :],
                              zc[kt][:, tok0:tok0 + T]) for kt in range(KT)])
            ksb = work.tile([P, T], BF16, tag="ksb")
            nc.vector.tensor_copy(ksb[:], ps[:])
            nc.sync.dma_start(out=kv_b[mt * P:(mt + 1) * P, 0:T], in_=ksb[:])
        tchunk = min(128, T)
        for ci in range(_cdiv(T, tchunk)):
            t0 = ci * tchunk
            ntk = min(tchunk, T - t0)
            ps = psp.tile([tchunk, C], F32, tag=mm_tag())
            mm_group(ps[0:ntk, :],
                     [(zc[kt][:, tok0 + t0:tok0 + t0 + ntk], wv_t[:, kt, :])
                      for kt in range(KT)])
            vsb = work.tile([tchunk, C], BF16, tag="vsb")
            nc.vector.tensor_copy(vsb[0:ntk, :], ps[0:ntk, :])
            nc.sync.dma_start(out=kv_b[C + t0:C + t0 + ntk, :],
                              in_=vsb[0:ntk, :])

        kv_g = dram.tile([NSH * (C + T), C], BF16, tag="kv_out")
        nc.gpsimd.collective_compute(
            "AllGather", mybir.AluOpType.bypass,
            replica_groups=[[0, 1, 2, 3], [4, 5, 6, 7]],
            ins=[kv_b.opt()], outs=[kv_g.opt()])

        agx3 = stats_send(x3pk, "sm0")
        red_fin(x3, agx3, 2)
        tap("x3", x3)
        x3f = [t[:].rearrange("p a b -> p (a b)") for t in x3]
        zx = ln_norm(x3f, ("z0", "z1"), bf=True)
        tap("zx", zx)

        # q in ch-layout [cout, E] (bf16); overlaps the kv AllGather
        q_ch = []
        for mt in range(MT):
            ps = psp.tile([P, g.E], F32, tag=mm_tag())
            mm_group(ps[:], [(wq_t[:, kt * MT + mt, :], zx[kt][:])
                             for kt in range(KT)])
            qt = mega.tile([P, g.E], BF16, tag=f"q{mt}")
            nc.scalar.activation(out=qt[:], in_=ps[:],
                                 func=AF.Identity, bias=acol('bq', mt))
            q_ch.append(qt)
        tap("q", q_ch)

        k_ch = [mega.tile([P, g.N], BF16, tag=f"midX{gi}", name=f"kch{gi}")
                for gi in range(KT)]
        v_tok = mega.tile([g.KJT, g.NT, C], BF16, tag="bigX1")
        for r in range(NSH):
            base = r * (C + T)
            for gi in range(KT):
                nc.sync.dma_start(
                    out=k_ch[gi][:, r * T:(r + 1) * T],
                    in_=kv_g[base + gi * P:base + (gi + 1) * P, 0:T])
            t0 = 0
            while t0 < T:
                tok = r * T + t0
                tile_i, prow = tok // g.KJT, tok % g.KJT
                cnt = min(g.KJT - prow, T - t0)
                nc.sync.dma_start(
                    out=v_tok[prow:prow + cnt, tile_i, :],
                    in_=kv_g[base + C + t0:base + C + t0 + cnt, :])
                t0 += cnt

        # attention: transposed scores (no max-sub; scores are O(1) here).
        # mask-muls run on the (idle) Pool engine; the AV output is
        # normalized by 1/sum instead of the NT prob tiles.
        attn = [mega.tile([P, g.E], F32, tag=("z2", "z3")[gi],
                          name=f"attn{gi}")
                for gi in range(KT)]
        for gi in range(KT):
            for hh in range(4):
                h = gi * 4 + hh
                hsl = slice(hh * HD, (hh + 1) * HD)
                e_h = mega.tile([g.KJT, g.NT, g.E], BF16, tag=f"T{h % 2}")
                for j in range(g.NT):
                    sps = psp.tile([g.KJT, g.E], F32,
                                   tag=("att0", "att1")[j % 2])
                    nc.tensor.matmul(
                        sps[:],
                        k_ch[gi][hsl, j * g.KJT:(j + 1) * g.KJT],
                        q_ch[gi][hsl, :],
                        start=True, stop=True,
                        tile_position=(hh * HD, 0))
                    nc.scalar.activation(out=e_h[:, j, :], in_=sps[:],
                                         func=AF.Exp, scale=att_scale)
                    nc.vector.tensor_mul(e_h[:, j, :],
                                         e_h[:, j, :], kmask[:, j, :])
                ssum = psp.tile([1, g.E], F32, tag="attS")
                for j in range(g.NT):
                    nc.tensor.matmul(ssum[:],
                                     ones_col_b[0:g.KJT, :],
                                     e_h[:, j, :],
                                     start=(j == 0), stop=(j == g.NT - 1))
                s_sb = stat1.tile([1, g.E], F32, tag="s_sb")
                nc.vector.tensor_copy(s_sb[:].bitcast(F32R), ssum[:])
                with nc.allow_low_precision(
                        reason="f32r feed to PE broadcast"):
                    nc.vector.reciprocal(s_sb[:].bitcast(F32R), s_sb[:])
                # broadcast 1/sum over the head's 32 out partitions
                rb_full = psp.tile([g.KJT, g.E], F32,
                                   tag="mm2" if hh % 2 == 0 else "mm3")
                nc.tensor.matmul(rb_full[:],
                                 ones_row[:, 0:g.KJT].bitcast(F32R),
                                 s_sb[:].bitcast(F32R),
                                 start=True, stop=True)
                # full-width av: head h's channels land on partitions
                # [32h, 32h+32); normalize + bias just that slice.
                av_ps = psp.tile([P, g.E], F32,
                                 tag="mm0" if hh % 2 == 0 else "mm1")
                for j in range(g.NT):
                    nc.tensor.matmul(
                        av_ps[:],
                        v_tok[:, j, gi * P:(gi + 1) * P],
                        e_h[:, j, :],
                        start=(j == 0), stop=(j == g.NT - 1))
                hs = slice(hh * HD, (hh + 1) * HD)
                avn = work.tile([P, g.E], F32, tag="avn")
                nc.scalar.copy(out=avn[hs, :], in_=av_ps[hs, :])
                nc.vector.tensor_mul(avn[hs, :], avn[hs, :],
                                     rb_full[hs, :])
                nc.scalar.activation(out=attn[gi][hs, :].bitcast(F32R),
                                     in_=avn[hs, :], func=AF.Identity,
                                     bias=acol('bv', gi)[hs, :])
        tap("kch", k_ch)
        tap("attn", attn)

        # LN + out-proj -> bottleneck (reuse q weight slot for proj weights)
        za = ln_norm([t[:] for t in attn], ("z0", "z1"))
        wp_t = mega.tile([P, KT * MT, P], F32, tag="wa1")
        nc.sync.dma_start(out=wp_t[:].bitcast(F32R),
                          in_=wp_d.rearrange("t p q -> p t q").bitcast(F32R))
        bott = alloc_padded(("sm0", "sm1"), nb, g.Wr, 1)
        for mt in range(MT):
            ps = psp.tile([P, g.E], F32, tag=mm_tag())
            mm_group(ps[:], [(wp_t[:, kt * MT + mt, :], za[kt][:])
                             for kt in range(KT)])
            nc.scalar.activation(
                out=bott[mt][:, :, 1:g.Wr + 1],
                in_=ps[:].rearrange("p (a b) -> p a b", a=nb),
                func=AF.Identity, bias=acol('bp', mt))
        edge_mask(bott, 'bott', g.bands('bott', g.Hr))
        tap("bott", bott)

        # ---------------------------------------------------- up stack
        def tconv_stage(src, out_st, in_st, wst, n_out, Wo, out_dsts,
                        align=None, parity_done=None, act_copies=False):
            """out_dsts(r, rx, mt, t0, ntr, Wh) -> strided dst AP.
            src: KT padded tiles. Loads per-class weights into tag wB.
            align: cap chunks at multiples of align (t-space), for outputs
            split into row-half tiles."""
            ybases = g.tconv_ybase(out_st, in_st)
            n_half = n_out // 2
            Wh = Wo // 2
            rpb = max(1, 512 // Wh)
            for r in range(2):
                pa, ybase = ybases[r]
                taps_y = tconv_taps(pa)
                for rx in range(2):
                    taps_x = tconv_taps(rx)
                    wcls = mega.tile([P, 4 * KT * MT, P], BF16,
                                     tag=f"wB{(r * 2 + rx) % 2}")
                    nc.sync.dma_start(
                        out=wcls[:],
                        in_=wup_d[wst, r * 2 + rx].rearrange(
                            "t p q -> p t q"))
                    t0 = 0
                    while t0 < n_half:
                        ntr = min(rpb, n_half - t0)
                        if align is not None:
                            ntr = min(ntr, align - t0 % align)
                        for mt in range(MT):
                            ps = psp.tile([P, rpb, Wh], F32, tag=mm_tag())
                            pairs = []
                            for iy, (ky, dy) in enumerate(taps_y):
                                for ix, (kx, dx) in enumerate(taps_x):
                                    for kt in range(KT):
                                        idx = ((iy * 2 + ix) * KT + kt) \
                                            * MT + mt
                                        ry0 = t0 + ybase + dy
                                        rhs = src[kt][:, ry0:ry0 + ntr,
                                                      1 + dx:1 + dx + Wh]
                                        pairs.append((wcls[:, idx, :], rhs))
                            mm_group_bf(ps[:, 0:ntr, :], pairs)
                            dst = out_dsts(r, rx, mt, t0, ntr, Wh)
                            if act_copies:
                                ce = copy_act
                            else:
                                ce = copy_act if (r + rx) % 2 == 0 \
                                    else copy_dve
                            ce(out=dst, in_=ps[:, 0:ntr, :])
                        t0 += ntr
                if parity_done is not None:
                    parity_done(r)

        # tconv1
        t1 = alloc_padded(("midX0", "midX1"), nu1, g.W2, 1)

        def dst_t1(r, rx, mt, t0, ntr, Wh):
            return t1[mt][:, r + 2 * t0:r + 2 * (t0 + ntr - 1) + 1:2,
                          1 + rx:1 + rx + 2 * Wh - 1:2]

        tconv_stage(bott, 'u1', 'bott', 0, nu1, g.W2, dst_t1)
        pk = statp.tile([P, 2 * KT], F32, tag="packu1")
        ol = g.owned_local('u1')
        for kt in range(KT):
            bn_stats_pack([t1[kt][:, ol:ol + g.H2 // NSH, 1:g.W2 + 1]],
                          kt, (g.H2 // NSH) * g.W2, pk, tg="u1")
        bo = stats_send(pk, "u1")
        a, b = stats_recv(bo, 2 * g.H2 * g.W2, 'up_g', 0)
        bn_apply(t1, lambda kt: [t1[kt][:, :, 1:g.W2 + 1]], a, b)
        edge_mask(t1, 'u1', g.bands('u1', g.H2))
        tap("t1", t1)

        # tconv2
        t2 = alloc_padded(("bigX0", "bigX1"), nu2, g.W1, 1)

        def dst_t2(r, rx, mt, t0, ntr, Wh):
            return t2[mt][:, r + 2 * t0:r + 2 * (t0 + ntr - 1) + 1:2,
                          1 + rx:1 + rx + 2 * Wh - 1:2]

        tconv_stage(t1, 'u2', 'u1', 1, nu2, g.W1, dst_t2)
        pk = statp.tile([P, 2 * KT], F32, tag="packu2")
        ol = g.owned_local('u2')
        for kt in range(KT):
            bn_stats_pack([t2[kt][:, ol:ol + g.H1 // NSH, 1:g.W1 + 1]],
                          kt, (g.H1 // NSH) * g.W1, pk, tg="u2")
        bo = stats_send(pk, "u2")
        a, b = stats_recv(bo, 2 * g.H1 * g.W1, 'up_g', 1)
        bn_apply(t2, lambda kt: [t2[kt][:, :, 1:g.W1 + 1]], a, b)
        edge_mask(t2, 'u2', g.bands('u2', g.H1))
        tap("t2", t2)

        # tconv3: output halves in bf16 on the T-tags
        t3 = [[mega.tile([P, nu3h, W], BF16, tag=f"T{2 * kt + hf}",
                         name=f"t3_{kt}_{hf}")
               for hf in range(2)] for kt in range(KT)]

        def dst_t3(r, rx, mt, t0, ntr, Wh):
            # rows r+2*t0 .. step2, count ntr; may straddle halves only if
            # chunk crosses nu3h -> chunk rows are within one half by rpb
            row0 = r + 2 * t0
            hf = row0 // nu3h
            assert (r + 2 * (t0 + ntr - 1)) // nu3h == hf
            lr = row0 - hf * nu3h
            return t3[mt][hf][:, lr:lr + 2 * (ntr - 1) + 1:2,
                              rx:rx + 2 * Wh - 1:2]

        # u3 stats split by output-row parity: parity-0 rows' bn_stats run
        # while the parity-1 tconv3 classes are still on the PE
        st_u3 = statp.tile([P, KT * nu3, 6], F32, tag="bnst3")

        def u3_parity_stats(r):
            for kt in range(KT):
                for hf in range(2):
                    for lr in range(r, nu3h, 2):
                        row = kt * nu3 + hf * nu3h + lr
                        nc.vector.bn_stats(out=st_u3[:, row, :],
                                           in_=t3[kt][hf][:, lr, :])

        tconv_stage(t2, 'u3', 'u2', 2, nu3, W, dst_t3, align=nu3h // 2,
                    parity_done=u3_parity_stats, act_copies=True)
        pk = statp.tile([P, 2 * KT], F32, tag="packu3")
        nloc_u3 = float((g.H // NSH) * W)
        for kt in range(KT):
            mv = statp.tile([P, 2], F32, tag="bnmv")
            nc.vector.bn_aggr(out=mv[:],
                              in_=st_u3[:, kt * nu3:(kt + 1) * nu3, :])
            t1s = work.tile([P, 1], F32, tag="bn_t1")
            nc.vector.tensor_mul(t1s[:], mv[:, 0:1], mv[:, 0:1])
            nc.vector.tensor_add(t1s[:], t1s[:], mv[:, 1:2])
            nc.scalar.mul(out=pk[:, kt:kt + 1], in_=mv[:, 0:1], mul=nloc_u3)
            nc.scalar.mul(out=pk[:, KT + kt:KT + kt + 1], in_=t1s[:],
                          mul=nloc_u3)

        # final: BN+ReLU + residual + store. Residual rows (bf16) prefetch
        # BEFORE the stats collective so the DMA queue isn't serialized
        # behind the stats readback; the add runs on the idle Pool engine.
        res_lo = g.owned_local('r0')
        rchunk = 8
        nch = _cdiv(nu3, rchunk)
        order = [(kt, ci) for kt in range(KT) for ci in range(nch)]

        RES_TAGS = ("bigC0", "bigC1", "midC0", "midC1")

        def res_load(kt, ci, slot):
            y = ci * rchunk
            nr = min(rchunk, nu3 - y)
            t = mega.tile([P, rchunk, W], BF16, tag=RES_TAGS[slot % 4])
            nc.sync.dma_start(
                out=t[:, 0:nr, :],
                in_=xin[kt, :, res_lo + y:res_lo + y + nr, 1:W + 1])
            return t

        res_t = {}
        for i, (kt, ci) in enumerate(order[:4]):
            res_t[(kt, ci)] = res_load(kt, ci, i)
        bo = stats_send(pk, "u3")
        a, b = stats_recv(bo, 2 * g.H * g.W, 'up_g', 2)
        for i, (kt, ci) in enumerate(order):
            y = ci * rchunk
            nr = min(rchunk, nu3 - y)
            hf, lr = y // nu3h, y % nu3h
            res = res_t.pop((kt, ci))
            osb = mega.tile([P, rchunk, W], BF16,
                            tag=("wB0", "wB1", "wA0", "wA1")[i % 4])
            nc.scalar.activation(out=osb[:, 0:nr, :],
                                 in_=t3[kt][hf][:, lr:lr + nr, :],
                                 func=AF.Relu,
                                 scale=a[:, kt:kt + 1],
                                 bias=b[:, kt:kt + 1])
            nc.vector.tensor_add(osb[:, 0:nr, :], osb[:, 0:nr, :],
                                 res[:, 0:nr, :])
            nc.sync.dma_start(out=out_d[kt, :, y:y + nr, :],
                              in_=osb[:, 0:nr, :])
            if i + 4 < len(order):
                kt2, ci2 = order[i + 4]
                res_t[(kt2, ci2)] = res_load(kt2, ci2, i + 4)

    nc.compile()
    return nc, g


# ---------------------------------------------------------------- host side
def _pad_rows(arr, lo, hi, H):
    n = hi - lo + 1
    out = np.zeros((arr.shape[0], n, arr.shape[2]), np.float32)
    a0, a1 = max(lo, 0), min(hi + 1, H)
    if a1 > a0:
        out[:, a0 - lo:a1 - lo, :] = arr[:, a0:a1, :]
    return out


def _prep_shared(inputs, g):
    red_w = [np.asarray(w, np.float32) for w in inputs['red_w']]
    up_w = [np.asarray(w, np.float32) for w in inputs['up_w']]
    wred = np.zeros((3, 9 * KT * MT, P, P), np.float32)
    for st in range(3):
        for ky in range(3):
            for kx in range(3):
                for kt in range(KT):
                    for mt in range(MT):
                        idx = ((ky * 3 + kx) * KT + kt) * MT + mt
                        wred[st, idx] = red_w[st][mt * P:(mt + 1) * P,
                                                  kt * P:(kt + 1) * P,
                                                  ky, kx].T
    # tconv weights grouped by parity class in the kernel's tap order
    wup = np.zeros((3, 4, 4 * KT * MT, P, P), np.float32)
    stage_out_in = (('u1', 'bott'), ('u2', 'u1'), ('u3', 'u2'))
    for st in range(3):
        ybases = g.tconv_ybase(*stage_out_in[st])
        for r in range(2):
            pa, _ = ybases[r]
            taps_y = tconv_taps(pa)
            for rx in range(2):
                taps_x = tconv_taps(rx)
                for iy, (ky, dy) in enumerate(taps_y):
                    for ix, (kx, dx) in enumerate(taps_x):
                        for kt in range(KT):
                            for mt in range(MT):
                                idx = ((iy * 2 + ix) * KT + kt) * MT + mt
                                wup[st, r * 2 + rx, idx] = \
                                    up_w[st][mt * P:(mt + 1) * P,
                                             kt * P:(kt + 1) * P,
                                             ky, kx].T

    def eff(wname, gname, bname):
        wv = np.asarray(inputs[wname], np.float32)
        gv = np.asarray(inputs[gname], np.float32)
        bv = np.asarray(inputs[bname], np.float32)
        return wv * gv[None, :], wv @ bv

    wq_e, bq_e = eff('wq', 'lnq_g', 'lnq_b')
    wk_e, _ = eff('wk', 'lnk_g', 'lnk_b')   # k bias cancels in softmax
    wv_e, bv_e = eff('wv', 'lnv_g', 'lnv_b')
    wp_e, bp_e = eff('proj_w', 'lno_g', 'lno_b')
    bp_e = bp_e + np.asarray(inputs['proj_b'], np.float32)

    def pack4(w):
        o = np.zeros((KT * MT, P, P), np.float32)
        for kt in range(KT):
            for mt in range(MT):
                o[kt * MT + mt] = w[mt * P:(mt + 1) * P,
                                    kt * P:(kt + 1) * P].T
        return o

    wv_t = np.zeros((KT, P, C), np.float32)
    for kt in range(KT):
        wv_t[kt] = wv_e.T[kt * P:(kt + 1) * P, :]

    shared = dict(wred=wred.astype(NPBF), wup=wup.astype(NPBF),
                  wq_t=pack4(wq_e), wk_t=pack4(wk_e),
                  wv_t=wv_t, wp_t=pack4(wp_e),
                  ones_row=np.ones((1, P), np.float32),
                  onesc=np.ones((P, 1), np.float32),
                  zpad=np.zeros((P, P), NPBF))
    return shared, (bq_e, bv_e, bp_e)


def _prep_aux(inputs, g, s, biases):
    bq_e, bv_e, bp_e = biases
    aux = np.zeros((P, NAUX), np.float32)
    for st in range(3):
        for kt in range(KT):
            sl = slice(kt * P, (kt + 1) * P)
            aux[:, AUX[('red_g', st, kt)]] = np.asarray(inputs['red_g'][st])[sl]
            aux[:, AUX[('red_beta', st, kt)]] = \
                np.asarray(inputs['red_beta'][st])[sl]
            aux[:, AUX[('up_g', st, kt)]] = np.asarray(inputs['up_g'][st])[sl]
            aux[:, AUX[('up_beta', st, kt)]] = \
                np.asarray(inputs['up_beta'][st])[sl]
    for mt in range(MT):
        sl = slice(mt * P, (mt + 1) * P)
        aux[:, AUX[('bq', mt)]] = bq_e[sl]
        aux[:, AUX[('bv', mt)]] = bv_e[sl]
        aux[:, AUX[('bp', mt)]] = bp_e[sl]
    stage_h = {'r1': g.H1, 'r2': g.H2, 'bott': g.Hr, 'u1': g.H2, 'u2': g.H1}
    for name, (tb, bb) in BANDS.items():
        al, lo, hi = getattr(g, name)
        n = hi - lo + 1
        Hst = stage_h[name]
        for r in range(tb):
            ab = al * s + lo + r
            aux[:, AUX[(name, 't', r)]] = 0.0 if ab < 0 else 1.0
        for r in range(bb):
            ab = al * s + lo + (n - bb + r)
            aux[:, AUX[(name, 'b', r)]] = 0.0 if ab > Hst - 1 else 1.0
    aux[:, AUX['ones']] = 1.0
    return aux


def _prep_kmask(g, s):
    ranges = query_kv_ranges(g.Hr, g.Wr)
    m = np.zeros((g.N, g.E), np.float32)
    tok_base = (g.bott[0] * s + g.bott[1]) * g.Wr
    for e in range(g.E):
        t = min(max(tok_base + e, 0), g.N - 1)
        lo, hi = ranges[t]
        m[lo:hi, e] = 1.0
    return m.reshape(g.NT, g.KJT, g.E)


def _prep_core(inputs, g, b, s, biases):
    x = np.asarray(inputs['x'], np.float32)
    ctx = np.asarray(inputs['context'], np.float32)
    H, W = g.H, g.W
    al, lo, hi = g.r0
    out = {}
    for nm, src in (('xin', x[b]), ('cin', ctx[b])):
        sl = _pad_rows(src, al * s + lo, al * s + hi, H)
        padded = np.zeros((C, sl.shape[1], W + 2), np.float32)
        padded[:, :, 1:W + 1] = sl
        out[nm] = np.ascontiguousarray(
            padded.reshape(KT, P, sl.shape[1], W + 2)).astype(NPBF)
    out['aux'] = _prep_aux(inputs, g, s, biases)
    out['kmask'] = np.ascontiguousarray(_prep_kmask(g, s)).astype(NPBF)
    return out


_CACHE = {}


def _get_built(H, W):
    key = (H, W)
    if key not in _CACHE:
        _CACHE[key] = build(H, W)
    return _CACHE[key]


def make_in_maps(inputs, g):
    shared, biases = _prep_shared(inputs, g)
    in_maps = []
    for core in range(NCORES):
        b, s = core // NSH, core % NSH
        m = dict(shared)
        m.update(_prep_core(inputs, g, b, s, biases))
        in_maps.append(m)
    return in_maps


def assemble_out(results, g, B=2):
    H, W = g.H, g.W
    out = np.zeros((B, C, H, W), np.float32)
    for core in range(NCORES):
        b, s = core // NSH, core % NSH
        o = results[core]['out'].astype(np.float32).reshape(C, g.n['u3'], W)
        out[b, :, (H // NSH) * s:(H // NSH) * (s + 1), :] = o
    return out


def kernel(**inputs):
    x = np.asarray(inputs['x'])
    H, W = x.shape[2], x.shape[3]
    nc, g = _get_built(H, W)
    in_maps = make_in_maps(inputs, g)
    res = run_bass_kernel_spmd(nc, in_maps, core_ids=list(range(NCORES)))
    return assemble_out(res.results, g)

